# revision 40
# baseline (speedup 1.0000x reference)
"""Trainium2 Bass kernel for nn_Attention_19404662243470.

Sharding: 8 cores = (batch 2) x (heads 4). Each core computes the full
attention pipeline for its (b, h) pair in transposed layout [d, n]; the
final pointwise conv partials are ReduceScattered per 512-position chunk
(8 small RS ops pipelined inside the attention loop) within each batch's
4-core group, and LayerNorm2d runs inline on each RS chunk as it lands.

Key layout/speed choices:
 - everything bf16 on the PE paths (FWL weight loads, halved DMA bytes);
   fp32 only inside PSUM and the LN scalar chains. LN broadcast/stat
   matmuls also run bf16 (1-pass) instead of fp32 (2-pass HIGH mode).
 - q/k pointwise packed into one [128ch -> 128] matmul (q rows 0-63,
   k rows 64-127 of a combined QKRAW tile).
 - q/k LN stats accumulate across all 8 chunks into ONE psum bank via
   4-way column tiling (mu_q@p0, e2_q@p32, mu_k@p64, e2_k@p96); squares
   feeding E[x^2] run on GpSimd in bf16; LN chains run in-psum.
 - softmax runs without max-subtraction; denominator falls out of the AV
   matmul via an appended ones-row in V.
 - exp is split across engines: ACT does 10 of 16 j-groups per chunk,
   DVE does 6 via the Schraudolph int-trick (x*A+B -> int32, whose top
   16 bits ARE the bf16 exp); the AV matmul reads the int32 tile through
   a stride-2 bf16 view.
 - out-LN uses scale invariance: LN(num/den + v) == LN(num + den*v), so
   no reciprocal is ever computed.
 - q/k halves are mirrored to partitions 64-127 with SBUF->SBUF DMA
   (row packing doubles S^T throughput; contraction is only 64 deep).
 - input DMAs are chunked so the depthwise conv starts ~2us in.
"""

import numpy as np

import concourse.bass as bass
import concourse.tile as tile
from concourse import bacc, mybir
from concourse.bass_utils import run_bass_kernel_spmd

dt = mybir.dt
AF = mybir.ActivationFunctionType
OP = mybir.AluOpType

B, DIM, Hs, Ws = 2, 128, 64, 64
HEADS, DH = 4, 64
N = Hs * Ws  # 4096
EPS = 1e-6
IC = 512  # i-chunk width
NIC = N // IC  # 8
JB = 128  # j-block
NJB = N // JB  # 32
NCH = N // 128  # 32
GR = Hs + 2  # 66 grid rows
GC = Hs + 4  # 68 grid cols (interior at col 2 -> 4B-aligned bf16 rows)

# Schraudolph fast-exp: exp(x/8) ~= bf16_bits(int32(x*EXA + EXB) >> 16)
EXA = float((2.0**23) / np.log(2.0) / 8.0)
EXB = float(127 * 2**23 - 335000.0)

_TABLES_PATCHED = False


def _patch_act_tables():
    """Restrict Exp/Ln to the natural_log_exp_and_others set so the ACT
    table never reloads between the softmax Exp stream and the LN-chain
    Ln/Exp pairs (a reload costs ~2.7us and stalls the exp feed)."""
    global _TABLES_PATCHED
    if _TABLES_PATCHED:
        return
    from concourse import bacc as _bacc_mod

    orig = _bacc_mod.get_activation_tables

    def patched(arch):
        tabs = dict(orig(arch))
        keep = {mybir.ActivationFunctionType.Exp, mybir.ActivationFunctionType.Ln}
        return {
            name: (fns if name == "natural_log_exp_and_others" else fns - keep)
            for name, fns in tabs.items()
        }

    _bacc_mod.get_activation_tables = patched
    _TABLES_PATCHED = True


def _build():
    _patch_act_tables()
    nc = bacc.Bacc()

    def par(name, shape, dtyp=dt.float32):
        return nc.declare_dram_parameter(name, list(shape), dtyp, isOutput=False)

    x = par("x", [DIM, GR * GC], dt.bfloat16)  # pre-padded grid layout
    dwpos = par("dwpos", [DIM, N], dt.bfloat16)  # dw3x3(pos), CPU-precomputed
    qtaps = par("qtaps", [DIM, 9])  # dw taps; diag built on-chip
    qkpw = par("qkpw", [DIM, 128], dt.bfloat16)  # [pwq | pwk]
    pwv = par("pwv", [DIM, DH], dt.bfloat16)
    o8sel = par("o8sel", [128, 8 * 8], dt.bfloat16)  # slice c: [128,8], col c=1/64
    w8q = par("w8q", [8, 8 * DH], dt.bfloat16)  # slice c: [8,64], row c=nq_w
    w8k = par("w8k", [8, 8 * DH], dt.bfloat16)
    lnqb = par("lnqb", [DH, 1])
    lnkb = par("lnkb", [DH, 1])
    lnow = par("lnow", [1, DH], dt.bfloat16)
    lnob = par("lnob", [DH, 1])
    otaps = par("otaps", [DH, 9])
    opw = par("opw", [DH, DIM], dt.bfloat16)
    ln2w = par("ln2w", [1, DIM])
    ln2b = par("ln2b", [1, DIM])
    o64hd = par("o64h", [DH, 1], dt.bfloat16)
    onesrd = par("onesr", [1, DH], dt.bfloat16)
    out_ext = nc.declare_dram_parameter("out", [N // 4, DIM], dt.float32, isOutput=True)

    rs_in = nc.dram_tensor("rs_in", [N, DIM], dt.bfloat16)
    rs_out = nc.dram_tensor("rs_out", [N // 4, DIM], dt.bfloat16)

    with (
        nc.allow_low_precision(reason="bf16 compute by design"),
        tile.TileContext(nc) as tc,
        tc.tile_pool(name="main", bufs=1) as main,
        tc.tile_pool(name="tmp2", bufs=2) as tmp2,
    ):
        # ---- input DMAs first: they gate the depthwise conv ----
        Xg = main.tile([DIM, GR * GC], dt.bfloat16)
        DWPOS = main.tile([DIM, N], dt.bfloat16)
        qtaps_t = main.tile([DIM, 9], dt.float32)
        otaps_t = main.tile([DH, 9], dt.float32)
        nc.gpsimd.dma_start(out=qtaps_t, in_=qtaps[:, :])
        nc.gpsimd.dma_start(out=otaps_t, in_=otaps[:, :])
        # Xg in 4 row-pieces (small first piece so dwconv(0) starts early)
        xrows = [0, 11, 27, 43, 66]
        for i in range(4):
            q = nc.sync if i % 2 == 0 else nc.scalar
            q.dma_start(
                out=Xg[:, xrows[i] * GC : xrows[i + 1] * GC],
                in_=x[:, xrows[i] * GC : xrows[i + 1] * GC],
            )
        for i in range(2):
            nc.gpsimd.dma_start(
                out=DWPOS[:, i * (N // 2) : (i + 1) * (N // 2)],
                in_=dwpos[:, i * (N // 2) : (i + 1) * (N // 2)],
            )
        Xg = Xg.rearrange("p (r c) -> p r c", c=GC)
        # diag weight matrices built on-chip from the taps (saves ~370KB of
        # startup HBM traffic): identity mask once, then 9 per-tap muls each
        qdg = main.tile([DIM, 9, DIM], dt.bfloat16)
        odg = main.tile([DH, 9, DH], dt.bfloat16)
        idmask = main.tile([DIM, DIM], dt.bfloat16)
        nc.vector.memset(idmask, 1.0)
        nc.gpsimd.affine_select(
            out=idmask, in_=idmask, compare_op=OP.is_equal, fill=0.0,
            base=0, pattern=[[1, DIM]], channel_multiplier=-1,
        )
        for t in range(9):
            nc.vector.tensor_scalar_mul(
                out=qdg[:, t, :], in0=idmask, scalar1=qtaps_t[:, t : t + 1]
            )
            nc.vector.tensor_scalar_mul(
                out=odg[:, t, :], in0=idmask[0:DH, 0:DH],
                scalar1=otaps_t[:, t : t + 1],
            )

        # ---- persistent SBUF tiles ----
        QL = main.tile([128, N], dt.bfloat16)  # LN'd q, duplicated on both halves
        KL = main.tile([128, N], dt.bfloat16)
        QKRAW = main.tile([128, N], dt.bfloat16)  # rows 0-63 raw q, 64-127 raw k
        SCB8q = main.tile([8, 2 * IC], dt.bfloat16)  # q-LN rs | mu*rs
        SCB8k = main.tile([8, 2 * IC], dt.bfloat16)
        VT = main.tile([DH, N], dt.bfloat16)  # v^T for the skip connection
        V = main.tile([128, NCH, DH + 1], dt.bfloat16)
        SCB = main.tile([1, 2 * N], dt.bfloat16)  # attention out-LN: rs | mu*rs
        Og = main.tile([DH, GR, GC], dt.bfloat16)  # padded out-LN grid
        opw_t = main.tile([DH, DIM], dt.bfloat16)
        nc.scalar.dma_start(out=opw_t, in_=opw[:, :])
        DWO = main.tile([DH, N], dt.bfloat16)
        o64h = main.tile([DH, 1], dt.bfloat16)
        nc.sync.dma_start(out=o64h, in_=o64hd[:, :])
        o8sel_t = main.tile([128, 8, 8], dt.bfloat16)
        nc.sync.dma_start(out=o8sel_t, in_=o8sel[:, :].rearrange("p (c e) -> p c e", c=8))
        w8q_t = main.tile([8, 8, DH], dt.bfloat16)
        nc.sync.dma_start(out=w8q_t, in_=w8q[:, :].rearrange("p (c e) -> p c e", c=8))
        w8k_t = main.tile([8, 8, DH], dt.bfloat16)
        nc.sync.dma_start(out=w8k_t, in_=w8k[:, :].rearrange("p (c e) -> p c e", c=8))
        lnqb_t = main.tile([DH, 1], dt.float32)
        lnkb_t = main.tile([DH, 1], dt.float32)
        lnob_t = main.tile([DH, 1], dt.float32)
        nc.sync.dma_start(out=lnqb_t, in_=lnqb[:, :])
        nc.sync.dma_start(out=lnkb_t, in_=lnkb[:, :])
        nc.sync.dma_start(out=lnob_t, in_=lnob[:, :])
        lnow_t = main.tile([1, DH], dt.bfloat16)
        nc.sync.dma_start(out=lnow_t, in_=lnow[:, :])
        onesr = main.tile([1, DH], dt.bfloat16)
        nc.sync.dma_start(out=onesr, in_=onesrd[:, :])
        w_b = main.tile([128, DIM], dt.float32)
        b_b = main.tile([128, DIM], dt.float32)
        nc.scalar.dma_start(out=w_b, in_=ln2w[:, :].to_broadcast([128, DIM]))
        nc.scalar.dma_start(out=b_b, in_=ln2b[:, :].to_broadcast([128, DIM]))
        epsP = main.tile([128, 1], dt.float32)
        nc.vector.memset(epsP, EPS)
        nc.vector.memset(V, 1.0)
        nc.vector.memset(Og, 0.0)

        # ============ Stage A: dw conv + qkv pointwise + LN stats ============
        with tc.tile_pool(name="stageA", bufs=1) as pA, tc.tile_pool(
            name="psST", bufs=1, space="PSUM"
        ) as psST:
            psA1cm = tc.tile_pool(name="psA1", bufs=1, space="PSUM")
            psA1 = psA1cm.__enter__()
            pwv_t = pA.tile([DIM, DH], dt.bfloat16)
            qkpw_t = pA.tile([DIM, 128], dt.bfloat16)
            nc.sync.dma_start(out=pwv_t, in_=pwv[:, :])
            nc.sync.dma_start(out=qkpw_t, in_=qkpw[:, :])
            Yr = pA.tile([DIM, N], dt.bfloat16)
            # stats accumulator: mu_q@[0:8], e2_q@[32:40], mu_k@[64:72], e2_k@[96:104]
            ST8 = psST.tile([128, IC], dt.float32, tag="st8", bufs=1)

            def dwconv(c):
                dwp = psA1.tile([DIM, IC], dt.float32, tag="dw", bufs=2)
                r0 = c * 8
                t = 0
                for di in range(3):
                    for dj in range(3):
                        nc.tensor.matmul(
                            dwp,
                            qdg[:, t, :],
                            Xg[:, r0 + di : r0 + di + 8, 1 + dj : 1 + dj + Ws],
                            start=(t == 0),
                            stop=(t == 8),
                        )
                        t += 1
                nc.vector.tensor_add(
                    out=Yr[:, c * IC : (c + 1) * IC],
                    in0=DWPOS[:, c * IC : (c + 1) * IC],
                    in1=dwp,
                )

            def qkv_chunk(c):
                """pointwise q|k + v for chunk c, LN stats accumulate."""
                cs = slice(c * IC, (c + 1) * IC)
                qk = psA1.tile([128, IC], dt.float32, tag="qk", bufs=2)
                nc.tensor.matmul(qk, qkpw_t, Yr[:, cs], start=True, stop=True)
                if c % 2 == 0:
                    nc.scalar.copy(out=QKRAW[:, cs], in_=qk)
                else:
                    nc.vector.tensor_copy(out=QKRAW[:, cs], in_=qk)
                # v^T (skip connection) rides the same rhs
                vt = psA1.tile([DH, IC], dt.float32, tag="vt", bufs=1)
                nc.tensor.matmul(vt, pwv_t, Yr[:, cs], start=True, stop=True)
                if c % 2 == 1:
                    nc.scalar.copy(out=VT[:, cs], in_=vt)
                else:
                    nc.vector.tensor_copy(out=VT[:, cs], in_=vt)
                # squares on gpsimd (bf16), both halves in one op
                sq = tmp2.tile([128, IC], dt.bfloat16, tag="sq", bufs=2)
                nc.gpsimd.tensor_mul(out=sq, in0=QKRAW[:, cs], in1=QKRAW[:, cs])
                # 4-way col-tiled stats accumulation into ST8
                nc.tensor.matmul(
                    ST8[0:8, :], o8sel_t[0:DH, c, :], QKRAW[0:DH, cs],
                    start=(c == 0), stop=(c == NIC - 1), skip_group_check=True,
                    tile_position=(0, 0),
                )
                nc.tensor.matmul(
                    ST8[32:40, :], o8sel_t[0:DH, c, :], sq[0:DH, :],
                    start=(c == 0), stop=(c == NIC - 1), skip_group_check=True,
                    tile_position=(0, 32),
                )
                nc.tensor.matmul(
                    ST8[64:72, :], o8sel_t[DH:128, c, :], QKRAW[DH:128, cs],
                    start=(c == 0), stop=(c == NIC - 1), skip_group_check=True,
                    tile_position=(64, 64),
                )
                nc.tensor.matmul(
                    ST8[96:104, :], o8sel_t[DH:128, c, :], sq[DH:128, :],
                    start=(c == 0), stop=(c == NIC - 1), skip_group_check=True,
                    tile_position=(64, 96),
                )

            def vbuild(g, psp):
                """V ([pos, dh] layout) for chunk g (4 j-blocks)."""
                vp = psp.tile([128, 4 * DH], dt.float32, tag="vp", bufs=2)
                for t in range(4):
                    ch = 4 * g + t
                    nc.tensor.matmul(
                        vp[:, t * DH : (t + 1) * DH],
                        Yr[:, ch * 128 : (ch + 1) * 128],
                        pwv_t,
                        start=True,
                        stop=True,
                    )
                if g % 2 == 0:
                    nc.scalar.copy(
                        out=V[:, 4 * g : 4 * g + 4, 0:DH],
                        in_=vp.rearrange("p (t d) -> p t d", t=4),
                    )
                else:
                    nc.vector.tensor_copy(
                        out=V[:, 4 * g : 4 * g + 4, 0:DH],
                        in_=vp.rearrange("p (t d) -> p t d", t=4),
                    )

            # pipelined: dw(c) || qkv(c-1) || vbuild(c-1 for first half)
            dwconv(0)
            for c in range(1, NIC):
                dwconv(c)
                qkv_chunk(c - 1)
                if c - 1 < NIC // 2:
                    vbuild(c - 1, psA1)
            qkv_chunk(NIC - 1)
            psA1cm.__exit__(None, None, None)

            def chain8(mu, e2, scb8, biasrows):
                """in-psum LN chain on [8, IC] stat rows -> bf16 rs|mrs.
                (mu^2 via ACT Square: DVE cannot read two PSUM operands.)"""
                t = tmp2.tile([8, IC], dt.float32r, tag="ch8", bufs=2)
                nc.scalar.activation(out=t, in_=mu, func=AF.Square)
                nc.vector.tensor_sub(out=e2, in0=e2, in1=t)
                nc.scalar.activation(out=e2, in_=e2, func=AF.Ln, bias=biasrows)
                nc.scalar.activation(
                    out=scb8[:, 0:IC], in_=e2, func=AF.Exp, scale=-0.5
                )
                nc.vector.tensor_mul(
                    out=scb8[:, IC : 2 * IC], in0=mu, in1=scb8[:, 0:IC]
                )

            with tc.tile_pool(name="psA2", bufs=1, space="PSUM") as psA2:

                def applyk(c):
                    cs = slice(c * IC, (c + 1) * IC)
                    bc = psA2.tile([128, IC], dt.float32, tag="bck", bufs=2)
                    nc.tensor.matmul(
                        bc[0:DH, :], w8k_t[:, c, :], SCB8k[:, 0:IC],
                        start=True, stop=True,
                    )
                    nc.tensor.matmul(
                        bc[DH:128, :], w8k_t[:, c, :], SCB8k[:, IC : 2 * IC],
                        start=True, stop=True,
                    )
                    T = tmp2.tile([DH, IC], dt.bfloat16, tag="T")
                    nc.vector.tensor_mul(out=T, in0=QKRAW[DH:128, cs], in1=bc[0:DH, :])
                    nc.vector.scalar_tensor_tensor(
                        out=KL[0:DH, cs],
                        in0=T,
                        scalar=lnkb_t,
                        in1=bc[DH:128, :],
                        op0=OP.add,
                        op1=OP.subtract,
                    )
                    # per-chunk mirror so KL[64:128] completes with the applies
                    q = nc.sync if c % 2 == 0 else nc.scalar
                    q.dma_start(out=KL[DH:128, cs], in_=KL[0:DH, cs])

                def warm():
                    """Dependency-free dummy matmul that keeps the PE HAM
                    activity window busy through DVE-bound stretches."""
                    wp = psA2.tile([DH, DH], dt.float32, tag="warm", bufs=1)
                    nc.tensor.matmul(wp, onesr, onesr, start=True, stop=True)

                # v-build back half is pure-PE filler covering chain latency;
                # both chains run back-to-back (stats all ready together)
                vbuild(4, psA2)
                vbuild(5, psA2)
                chain8(ST8[64:72, :], ST8[96:104, :], SCB8k, epsP[0:8, :])
                chain8(ST8[0:8, :], ST8[32:40, :], SCB8q, epsP[0:8, :])
                vbuild(6, psA2)
                vbuild(7, psA2)
                warm()
                for c in range(NIC):
                    applyk(c)
                    warm()
                    warm()

        # ============ Stage B: attention with inline out-LN + RS + LN2d ============
        with tc.tile_pool(name="psB", bufs=1, space="PSUM") as psB, tc.tile_pool(
            name="sbB", bufs=3
        ) as sbB, tc.tile_pool(name="sbD", bufs=2) as pD:
            NG = NJB // 2  # 16 pair-groups per chunk
            DVE_G = (2, 4, 7, 9, 12, 14)  # groups whose exp runs on DVE
            pending_tail = []

            def apply_q(c):
                """q-LN apply for chunk c, pipelined inside the attention loop."""
                cs = slice(c * IC, (c + 1) * IC)
                bq = psB.tile([128, IC], dt.float32, tag="bq", bufs=1)
                nc.tensor.matmul(
                    bq[0:DH, :], w8q_t[:, c, :], SCB8q[:, 0:IC], start=True, stop=True
                )
                nc.tensor.matmul(
                    bq[DH:128, :], w8q_t[:, c, :], SCB8q[:, IC : 2 * IC],
                    start=True, stop=True,
                )
                T = tmp2.tile([DH, IC], dt.bfloat16, tag="Tq")
                nc.vector.tensor_mul(out=T, in0=QKRAW[0:DH, cs], in1=bq[0:DH, :])
                nc.vector.scalar_tensor_tensor(
                    out=QL[0:DH, cs],
                    in0=T,
                    scalar=lnqb_t,
                    in1=bq[DH:128, :],
                    op0=OP.add,
                    op1=OP.subtract,
                )
                nc.scalar.dma_start(out=QL[DH:128, cs], in_=QL[0:DH, cs])

            def attention_block(c, fills=None):
                """fills: {g: [callable]} — foreign work injected at group
                position g, where its upstream deps are already satisfied,
                so the in-order PE queue never stalls at chunk boundaries."""
                avp = psB.tile([DH + 1, IC], dt.float32, tag="avp", bufs=2)
                stgs = {}
                Es = {}

                def issue_st(g):
                    stg = psB.tile([128, 2 * IC], dt.float32, tag="stg", bufs=2)
                    j0 = 2 * g * JB
                    nc.tensor.matmul(
                        stg[:, 0:IC],
                        KL[0:DH, j0 : j0 + JB],
                        QL[0:DH, c * IC : (c + 1) * IC],
                        start=True,
                        stop=True,
                    )
                    nc.tensor.matmul(
                        stg[:, IC : 2 * IC],
                        KL[DH:128, j0 + JB : j0 + 2 * JB],
                        QL[DH:128, c * IC : (c + 1) * IC],
                        start=True,
                        stop=True,
                    )
                    stgs[g] = stg

                def issue_exp(g):
                    if g in DVE_G:
                        EI = sbB.tile([128, 2 * IC], dt.int32, tag="EI", bufs=2)
                        nc.vector.tensor_scalar(
                            out=EI,
                            in0=stgs.pop(g),
                            scalar1=EXA,
                            scalar2=EXB,
                            op0=OP.mult,
                            op1=OP.add,
                        )
                        Es[g] = EI.bitcast(dt.bfloat16).rearrange(
                            "p (a two) -> p a two", two=2
                        )
                    else:
                        E = sbB.tile([128, 2 * IC], dt.bfloat16, tag="E")
                        nc.scalar.activation(
                            out=E, in_=stgs.pop(g), func=AF.Exp, scale=float(DH**-0.5)
                        )
                        Es[g] = E

                def issue_av(g):
                    E = Es.pop(g)
                    for t in range(2):
                        jb = 2 * g + t
                        if g in DVE_G:
                            rhs = E[:, t * IC : (t + 1) * IC, 1:2]
                        else:
                            rhs = E[:, t * IC : (t + 1) * IC]
                        nc.tensor.matmul(
                            avp,
                            V[:, jb, :],
                            rhs,
                            start=(jb == 0),
                            stop=(jb == NJB - 1),
                            skip_group_check=True,
                        )

                issue_st(0)
                issue_exp(0)
                for g in range(1, NG):
                    issue_st(g)
                    issue_exp(g)
                    issue_av(g - 1)
                    if fills:
                        for f in fills.get(g, ()):
                            f()
                issue_av(NG - 1)

                # park numerator+denominator quickly to free avp (bufs=1: tail
                # runs during the NEXT chunk's matmuls, before its avp use)
                DEN = sbB.tile([1, IC], dt.bfloat16, tag="DEN", bufs=2)
                nc.vector.tensor_copy(out=DEN, in_=avp[DH : DH + 1, :])
                Tn = sbB.tile([DH, IC], dt.bfloat16, tag="Tn", bufs=2)
                nc.vector.tensor_copy(out=Tn, in_=avp[0:DH, :])
                return DEN, Tn

            tail_state = {}

            def tail_a(c, DEN, Tn):
                cs = slice(c * IC, (c + 1) * IC)
                # scale-invariant skip: OSc = num + den*v (LN output matches
                # LN(num/den + v) because LN normalizes per-position scale)
                tl = psB.tile([128, IC], dt.float32, tag="tl", bufs=1)
                nc.tensor.matmul(tl[0:DH, :], onesr, DEN, start=True, stop=True)
                OSc = sbB.tile([DH, IC], dt.bfloat16, tag="OS", bufs=2)
                nc.vector.tensor_mul(out=OSc, in0=VT[:, cs], in1=tl[0:DH, :])
                nc.gpsimd.tensor_add(out=OSc, in0=OSc, in1=Tn)
                sq = tmp2.tile([DH, IC], dt.bfloat16, tag="sqo", bufs=2)
                nc.gpsimd.tensor_mul(out=sq, in0=OSc, in1=OSc)
                tail_state[c] = (OSc, sq)

            def tail_b(c):
                OSc, sq = tail_state[c]
                # stats: mu@row0, e2@row32 of one bank (col-tiled pair)
                st = psB.tile([128, IC], dt.float32, tag="tl", bufs=1)
                nc.tensor.matmul(st[0:1, :], o64h, OSc, start=True, stop=True)
                nc.tensor.matmul(st[32:33, :], o64h, sq, start=True, stop=True)
                mu = st[0:1, :]
                e2 = st[32:33, :]
                # in-psum chain -> bf16 rs|mrs in SCB
                t = tmp2.tile([1, IC], dt.float32r, tag="cht", bufs=2)
                nc.scalar.activation(out=t, in_=mu, func=AF.Square)
                nc.vector.tensor_sub(out=e2, in0=e2, in1=t)
                nc.scalar.activation(out=e2, in_=e2, func=AF.Ln, bias=epsP[0:1, :])
                nc.scalar.activation(
                    out=SCB[:, c * IC : (c + 1) * IC], in_=e2, func=AF.Exp, scale=-0.5
                )
                nc.vector.tensor_mul(
                    out=SCB[:, N + c * IC : N + (c + 1) * IC],
                    in0=mu,
                    in1=SCB[:, c * IC : (c + 1) * IC],
                )

            def tail_c(c):
                OSc, _ = tail_state.pop(c)
                bc = psB.tile([128, IC], dt.float32, tag="tl", bufs=1)
                nc.tensor.matmul(
                    bc[0:DH, :], lnow_t, SCB[:, c * IC : (c + 1) * IC],
                    start=True, stop=True,
                )
                nc.tensor.matmul(
                    bc[DH:128, :], lnow_t, SCB[:, N + c * IC : N + (c + 1) * IC],
                    start=True, stop=True,
                )
                T = tmp2.tile([DH, IC], dt.float32, tag="T")
                nc.vector.tensor_mul(out=T, in0=OSc, in1=bc[0:DH, :])
                r0 = c * 8
                nc.vector.scalar_tensor_tensor(
                    out=Og[:, 1 + r0 : 9 + r0, 2 : 2 + Ws],
                    in0=T.rearrange("p (a b) -> p a b", b=Ws),
                    scalar=lnob_t,
                    in1=bc[DH:128, :].rearrange("p (a b) -> p a b", b=Ws),
                    op0=OP.add,
                    op1=OP.subtract,
                )

            def dw_a(c):
                dwpf = psB.tile([128, IC], dt.float32, tag="tl", bufs=1)
                dwp = dwpf[0:DH, :]
                r0 = c * 8
                t = 0
                for di in range(3):
                    for dj in range(3):
                        nc.tensor.matmul(
                            dwp,
                            odg[:, t, :],
                            Og[:, r0 + di : r0 + di + 8, 1 + dj : 1 + dj + Ws],
                            start=(t == 0),
                            stop=(t == 8),
                        )
                        t += 1
                nc.scalar.copy(out=DWO[:, c * IC : (c + 1) * IC], in_=dwp)

            def dw_b(c):
                pp = psB.tile([128, 4 * DIM], dt.float32, tag="tl", bufs=1)
                for t in range(4):
                    ch = 4 * c + t
                    nc.tensor.matmul(
                        pp[:, t * DIM : (t + 1) * DIM],
                        DWO[:, ch * 128 : (ch + 1) * 128],
                        opw_t,
                        start=True,
                        stop=True,
                    )
                PP = tmp2.tile([128, 4 * DIM], dt.bfloat16, tag="PP")
                nc.vector.tensor_copy(out=PP, in_=pp)
                for t in range(4):
                    ch = 4 * c + t
                    nc.sync.dma_start(
                        out=rs_in[ch * 128 : (ch + 1) * 128, :],
                        in_=PP[:, t * DIM : (t + 1) * DIM],
                    )
                nc.gpsimd.collective_compute(
                    "ReduceScatter",
                    OP.add,
                    replica_groups=[[0, 1, 2, 3], [4, 5, 6, 7]],
                    ins=[rs_in[c * IC : (c + 1) * IC, :]],
                    outs=[rs_out[c * JB : (c + 1) * JB, :]],
                )

            def stage_d(c):
                """LN2d on chunk c's RS result. Issued >=2 chunks after dw_b(c)
                so no queue ever blocks on an in-flight RS."""
                R = pD.tile([128, DIM], dt.bfloat16, tag="Rb")
                nc.sync.dma_start(out=R, in_=rs_out[c * JB : (c + 1) * JB, :])
                stb = pD.tile([128, 6], dt.float32, tag="stb")
                nc.vector.bn_stats(out=stb, in_=R)
                mv = pD.tile([128, 2], dt.float32, tag="mv")
                nc.vector.bn_aggr(out=mv, in_=stb)
                sd = pD.tile([128, 1], dt.float32, tag="sd")
                nc.scalar.activation(out=sd, in_=mv[:, 1:2], func=AF.Ln, bias=epsP)
                nc.scalar.activation(out=sd, in_=sd, func=AF.Exp, scale=-0.5)
                Rf = pD.tile([128, DIM], dt.float32, tag="R")
                nc.vector.tensor_scalar(
                    out=Rf,
                    in0=R,
                    scalar1=mv[:, 0:1],
                    scalar2=sd,
                    op0=OP.subtract,
                    op1=OP.mult,
                )
                R2 = pD.tile([128, DIM], dt.float32, tag="R2")
                nc.vector.tensor_mul(out=R2, in0=Rf, in1=w_b)
                nc.vector.tensor_add(out=R2, in0=R2, in1=b_b)
                nc.sync.dma_start(
                    out=out_ext[c * JB : (c + 1) * JB, :], in_=R2
                )

            apply_q(0)
            apply_q(1)
            for c in range(NIC):
                fills = {}
                if pending_tail:
                    cp, DENp, Tnp = pending_tail.pop()
                    fills[2] = [lambda cp=cp, D=DENp, T=Tnp: tail_a(cp, D, T)]
                    fills[7] = [lambda cp=cp: tail_b(cp)]
                    fills[10] = [lambda cp=cp: tail_c(cp)]
                if c + 2 < NIC:
                    fills[11] = [lambda cn=c + 2: apply_q(cn)]
                if c >= 2:
                    fills[12] = [lambda cd=c - 2: dw_a(cd)]
                    fills[14] = [lambda cd=c - 2: dw_b(cd)]
                if c >= 3:
                    fills[5] = [lambda cd=c - 3: stage_d(cd)]
                den_tn = attention_block(c, fills)
                pending_tail.append((c, *den_tn))
            cp, DENp, Tnp = pending_tail.pop()
            tail_a(cp, DENp, Tnp)
            tail_b(cp)
            tail_c(cp)
            dw_a(NIC - 2)
            dw_b(NIC - 2)
            dw_a(NIC - 1)
            dw_b(NIC - 1)
            for c in range(NIC - 3, NIC):
                stage_d(c)

    return nc


_cached = {}


def _get_nc():
    if "nc" not in _cached:
        nc = _build()
        nc.finalize()
        _cached["nc"] = nc
    return _cached["nc"]


def _make_in_maps(inputs):
    import ml_dtypes

    bf = ml_dtypes.bfloat16
    x = np.asarray(inputs["x"], np.float32)
    pe_w = np.asarray(inputs["pe_w"], np.float32)
    pe_b = np.asarray(inputs["pe_b"], np.float32)
    qkv_dw = np.asarray(inputs["qkv_dw"], np.float32)
    qkv_pw = np.asarray(inputs["qkv_pw"], np.float32)
    out_dw = np.asarray(inputs["out_dw"], np.float32)
    out_pw = np.asarray(inputs["out_pw"], np.float32)
    nq_w, nq_b = np.asarray(inputs["nq_w"], np.float32), np.asarray(
        inputs["nq_b"], np.float32
    )
    nk_w, nk_b = np.asarray(inputs["nk_w"], np.float32), np.asarray(
        inputs["nk_b"], np.float32
    )
    no_w, no_b = np.asarray(inputs["no_w"], np.float32), np.asarray(
        inputs["no_b"], np.float32
    )
    ln_w, ln_b = np.asarray(inputs["ln_w"], np.float32), np.asarray(
        inputs["ln_b"], np.float32
    )

    gx = np.linspace(0.0, 1.0, Hs, dtype=np.float64)
    gy = np.linspace(0.0, 1.0, Ws, dtype=np.float64)
    pos = (
        pe_w[:, 0:1, None] * gx[None, :, None]
        + pe_w[:, 1:2, None] * gy[None, None, :]
        + pe_b[:, None, None]
    )  # [DIM, H, W]
    posp = np.pad(pos, ((0, 0), (1, 1), (1, 1)))
    taps9 = qkv_dw.reshape(DIM, 9)
    dwpos = np.zeros((DIM, Hs, Ws), np.float64)
    t = 0
    for di in range(3):
        for dj in range(3):
            dwpos += posp[:, di : di + Hs, dj : dj + Ws] * taps9[:, t][:, None, None]
            t += 1
    dwpos = dwpos.reshape(DIM, N).astype(bf)

    idx = np.arange(DH)
    o8sel = np.zeros((DH, 8, 8), np.float32)
    for c in range(8):
        o8sel[:, c, c] = 1.0 / DH
    o8sel2 = np.concatenate([o8sel, o8sel], axis=0)  # both partition halves

    in_maps = []
    for core in range(8):
        b, h = core // 4, core % 4
        rows = h + HEADS * idx
        otaps = out_dw[rows].reshape(DH, 9)
        w8q = np.zeros((8, 8, DH), np.float32)
        w8k = np.zeros((8, 8, DH), np.float32)
        for c in range(8):
            w8q[c, c, :] = nq_w[h]
            w8k[c, c, :] = nk_w[h]
        xg = np.zeros((DIM, GR, GC), bf)
        xg[:, 1 : 1 + Hs, 2 : 2 + Ws] = x[b].reshape(DIM, Hs, Ws).astype(bf)
        qkpw = np.concatenate(
            [qkv_pw[rows, :].T, qkv_pw[DIM * 2 + rows, :].T], axis=1
        )  # [128, 128]
        m = {
            "x": np.ascontiguousarray(xg.reshape(DIM, GR * GC)),
            "dwpos": dwpos,
            "qtaps": np.ascontiguousarray(qkv_dw.reshape(DIM, 9)),
            "qkpw": np.ascontiguousarray(qkpw).astype(bf),
            "pwv": np.ascontiguousarray(qkv_pw[DIM * 4 + rows, :].T).astype(bf),
            "o8sel": np.ascontiguousarray(o8sel2.reshape(128, 64)).astype(bf),
            "w8q": np.ascontiguousarray(
                w8q.transpose(1, 0, 2).reshape(8, 8 * DH)
            ).astype(bf),
            "w8k": np.ascontiguousarray(
                w8k.transpose(1, 0, 2).reshape(8, 8 * DH)
            ).astype(bf),
            "lnqb": np.ascontiguousarray(nq_b[h][:, None]),
            "lnkb": np.ascontiguousarray(nk_b[h][:, None]),
            "lnow": np.ascontiguousarray(no_w[h][None, :]).astype(bf),
            "lnob": np.ascontiguousarray(no_b[h][:, None]),
            "otaps": np.ascontiguousarray(otaps),
            "opw": np.ascontiguousarray(out_pw[:, rows].T).astype(bf),
            "ln2w": np.ascontiguousarray(ln_w[None, :]),
            "ln2b": np.ascontiguousarray(ln_b[None, :]),
            "o64h": np.full((DH, 1), 1.0 / DH, np.float32).astype(bf),
            "onesr": np.ones((1, DH), np.float32).astype(bf),
        }
        in_maps.append(m)
    return in_maps


def run_on_device(inputs, **kw):
    nc = _get_nc()
    in_maps = _make_in_maps(inputs)
    res = run_bass_kernel_spmd(nc, in_maps, core_ids=list(range(8)), **kw)
    out = np.zeros((B, DIM, N), np.float32)
    for core in range(8):
        b, h = core // 4, core % 4
        o = res.results[core]["out"]  # rows: 8 chunks x 128 positions
        for c in range(NIC):
            g0 = c * IC + h * JB
            out[b][:, g0 : g0 + JB] = o[c * JB : (c + 1) * JB].T
    return out.reshape(B, DIM, Hs, Ws), res


def kernel(**inputs):
    out, _ = run_on_device(inputs)
    return out
